# revision 1
# baseline (speedup 1.0000x reference)
"""CircleLoss Trainium2 kernel (8-core SPMD).

Math: for S = cosine-sim(enc, dec) [N,N], both loss directions reduce to
per-wrapped-diagonal logsumexps of one matrix:
    out = mean_{d=1..N-1} softplus(L[d] + lse_p)
    L[d]  = log sum_j exp(g(S[j,(j+d)%N])),  g(s) = GAMMA*(max(s,-M)^2 - M^2)
    lse_p = logsumexp_j h(S[j,j]),           h(s) = -relu(1+M-s)*(s-(1-M))*GAMMA
g in [-4, 60] so sum(exp(g)) fits f32 with no max-pass.

Sharding: core r owns rows [1024r, 1024r+1024). Each core computes its
1024 x 8192 slab of S via PE matmuls (norms folded into operands), the
elementwise exp(g(.)) chain, bounces E tiles through a DRAM stripe and
reads them back with a sheared (diagonal) access pattern so wrapped
diagonals become columns, then bins per-diagonal sums with one-hot
ones-matmuls accumulated in PSUM. Host sums the 8 per-core [8192]
partials, adds the exact diagonal term, and finishes the tiny
softplus/mean in float64.
"""

import numpy as np

import concourse.bass as bass
import concourse.bacc as bacc
import concourse.mybir as mybir
from concourse.tile import TileContext
from concourse.masks import make_identity
from concourse.bass_utils import run_bass_kernel_spmd

N = 8192
D = 128
P = 128
NCORES = 8
R = N // NCORES          # 1024 rows per core
NBJ = R // P             # 8 row-tiles per core
F = 512                  # matmul free-dim chunk
WIN = 9216               # dec window columns per core (18 * 512)
NWB = WIN // P           # 72 window blocks
WS = WIN                 # stripe width (elements) per row-tile
SW = 17 * F              # written stripe width 8704
NWC = 16                 # 512-wide d-chunks
M_M = 0.25
GAMMA = 64.0
SQG = 8.0                # sqrt(GAMMA)
EXPB = -4.0              # -GAMMA*M^2
EPS = 1e-5

F32 = mybir.dt.float32
F16 = mybir.dt.float16
BF16 = mybir.dt.bfloat16

_CACHE = {}


def _build_program():
    nc = bacc.Bacc("TRN2", target_bir_lowering=False, debug=False,
                   num_devices=NCORES)
    enc_slab = nc.dram_tensor("enc_slab", [R, D], F32, kind="ExternalInput")
    dec_win = nc.dram_tensor("dec_win", [WIN, D], F32, kind="ExternalInput")
    acc_out = nc.dram_tensor("acc_out", [NWC, F], F32, kind="ExternalOutput")
    sdiag_out = nc.dram_tensor("sdiag_out", [P, NBJ], F32, kind="ExternalOutput")
    stripes = nc.dram_tensor("stripes", [NBJ, P, WS], BF16, kind="Internal")

    mx = mybir.AluOpType.max
    mul = mybir.AluOpType.mult
    add = mybir.AluOpType.add
    AF = mybir.ActivationFunctionType

    with TileContext(nc) as tc:
        with (
            tc.tile_pool(name="persist", bufs=1) as persist,
            tc.tile_pool(name="norms", bufs=1) as norms,
        ):
            dec_nT = persist.tile([P, WIN], BF16)
            enc_nT = persist.tile([P, R], BF16)
            onehot = persist.tile([P, NWC * NWC], BF16)
            expb = persist.tile([P, 1], F32)
            acc_sb = persist.tile([NWC, F], F32)
            nc.vector.memset(expb[:], EXPB)
            nc.gpsimd.memset(onehot[:], 0.0)
            for wc in range(NWC):
                nc.gpsimd.memset(onehot[:, wc * NWC + wc:wc * NWC + wc + 1], 1.0)

            dn2 = norms.tile([P, NWB], F32)
            dn_c = norms.tile([P, NWB], F32)
            inv_dn = norms.tile([P, NWB], F32)
            en2 = norms.tile([P, NBJ], F32)
            en_c = norms.tile([P, NBJ], F32)
            inv_en = norms.tile([P, NBJ], F32)
            dot_c = norms.tile([P, NBJ], F32)
            sd = norms.tile([P, NBJ], F32)

            # ---- prep ----
            with (
                tc.tile_pool(name="prep", bufs=2) as prep,
                tc.tile_pool(name="tpp", bufs=4) as tpp,
            ):
                dump = persist.tile([P, D], F32)
                # one big DMA each; tile[p, k, d] = src[128k + p, d]
                dw_all = prep.tile([P, NWB, D], F32, tag="dw_all")
                nc.sync.dma_start(
                    out=dw_all[:],
                    in_=bass.AP(tensor=dec_win, offset=0,
                                ap=[[D, P], [P * D, NWB], [1, D]]))
                eb_all = prep.tile([P, NBJ, D], F32, tag="eb_all")
                nc.sync.dma_start(
                    out=eb_all[:],
                    in_=bass.AP(tensor=enc_slab, offset=0,
                                ap=[[D, P], [P * D, NBJ], [1, D]]))

                dump2 = persist.tile([P, D], F32)

                # enc norms + normalize + transpose (small, do first)
                for k in range(NBJ):
                    nc.scalar.activation(dump[:], eb_all[:, k, :], AF.Square,
                                         accum_out=en2[:, k:k + 1])
                nc.scalar.activation(en_c[:], en2[:], AF.Sqrt)
                nc.vector.reciprocal(inv_en[:], en_c[:])
                for k in range(NBJ):
                    ebn = tpp.tile([P, D], BF16, tag="ebn")
                    nc.vector.tensor_scalar(out=ebn[:], in0=eb_all[:, k, :],
                                            scalar1=inv_en[:, k:k + 1], scalar2=None,
                                            op0=mul)
                    nc.sync.dma_start_transpose(enc_nT[:, k * P:(k + 1) * P], ebn[:])

                # dec norms + normalize + transpose, pipelined in groups of 8;
                # square-reduce split across ACT and DVE
                for g in range(NWB // NBJ):
                    for b in range(g * NBJ, (g + 1) * NBJ):
                        if b % 3 != 0:
                            nc.scalar.activation(dump[:], dw_all[:, b, :], AF.Square,
                                                 accum_out=dn2[:, b:b + 1])
                        else:
                            nc.vector.tensor_mul(dump2[:], dw_all[:, b, :],
                                                 dw_all[:, b, :])
                            nc.vector.tensor_reduce(dn2[:, b:b + 1], dump2[:],
                                                    mybir.AxisListType.X, add)
                    gs = slice(g * NBJ, (g + 1) * NBJ)
                    nc.scalar.activation(dn_c[:, gs], dn2[:, gs], AF.Sqrt)
                    nc.vector.reciprocal(inv_dn[:, gs], dn_c[:, gs])
                    for b in range(g * NBJ, (g + 1) * NBJ):
                        dwn = tpp.tile([P, D], BF16, tag="dwn")
                        nc.vector.tensor_scalar(out=dwn[:], in0=dw_all[:, b, :],
                                                scalar1=inv_dn[:, b:b + 1],
                                                scalar2=None, op0=mul)
                        nc.sync.dma_start_transpose(dec_nT[:, b * P:(b + 1) * P],
                                                    dwn[:])

                # diag dots + s_diag = dot / (en*dn + eps), exact
                for k in range(NBJ):
                    nc.vector.tensor_mul(dump2[:], eb_all[:, k, :], dw_all[:, k, :])
                    nc.vector.tensor_reduce(dot_c[:, k:k + 1], dump2[:],
                                            mybir.AxisListType.X, add)
                nc.vector.tensor_mul(sd[:], en_c[:, 0:NBJ], dn_c[:, 0:NBJ])
                nc.vector.tensor_scalar_add(sd[:], sd[:], EPS)
                nc.vector.reciprocal(sd[:], sd[:])
                nc.vector.tensor_mul(sd[:], sd[:], dot_c[:])
                nc.sync.dma_start(out=sdiag_out[:, :], in_=sd[:])

            # ---- main ----
            with (
                tc.tile_pool(name="mpsum", bufs=4, space="PSUM") as mpsum,
                tc.tile_pool(name="apsum", bufs=1, space="PSUM") as apsum,
                tc.tile_pool(name="chain", bufs=3) as chain,
                tc.tile_pool(name="stripe", bufs=2) as stripe_pool,
                tc.tile_pool(name="shear", bufs=4) as shear_pool,
            ):
                acc_ps = apsum.tile([NWC, F], F32)
                nc.vector.memset(acc_ps[:], 0.0)
                for bj in range(NBJ):
                    i_lo = bj // 4
                    ssb = stripe_pool.tile([P, SW], BF16, tag="ssb")
                    for i in range(17):
                        ic = i_lo + i
                        ps = mpsum.tile([P, F], F32, tag="ps")
                        nc.tensor.matmul(
                            ps[:], lhsT=enc_nT[:, bj * P:(bj + 1) * P],
                            rhs=dec_nT[:, ic * F:(ic + 1) * F],
                            start=True, stop=True)
                        g5 = chain.tile([P, F], F32, tag="g5")
                        nc.vector.tensor_scalar(out=g5[:], in0=ps[:],
                                                scalar1=-M_M, scalar2=SQG,
                                                op0=mx, op1=mul)
                        q5 = chain.tile([P, F], F16, tag="q5")
                        if i % 3 == 0:
                            nc.scalar.activation(q5[:], g5[:], AF.Square)
                        elif i % 3 == 1:
                            nc.vector.tensor_mul(q5[:], g5[:], g5[:])
                        else:
                            nc.gpsimd.tensor_mul(q5[:], g5[:], g5[:])
                        nc.scalar.activation(ssb[:, i * F:(i + 1) * F], q5[:],
                                             AF.Exp, bias=expb[:, 0:1], scale=1.0)
                    nc.sync.dma_start(
                        out=bass.AP(tensor=stripes, offset=bj * P * WS + i_lo * F,
                                    ap=[[WS, P], [1, SW]]),
                        in_=ssb[:])
                    for w2 in range(NWC // 2):
                        er = shear_pool.tile([P, 2 * F], BF16, tag="er")
                        nc.sync.dma_start(
                            out=er[:],
                            in_=bass.AP(tensor=stripes,
                                        offset=bj * P * WS + bj * P + w2 * 2 * F,
                                        ap=[[WS + 1, P], [1, 2 * F]]))
                        for h in range(2):
                            wc = 2 * w2 + h
                            nc.tensor.matmul(
                                acc_ps[:],
                                lhsT=onehot[:, wc * NWC:(wc + 1) * NWC],
                                rhs=er[:, h * F:(h + 1) * F],
                                start=False, stop=False,
                                skip_group_check=True)
                nc.scalar.copy(acc_sb[:], acc_ps[:])
                nc.sync.dma_start(out=acc_out[:, :], in_=acc_sb[:])
    nc.compile()
    return nc


def kernel(encoder_output: np.ndarray, decoder_output: np.ndarray) -> np.ndarray:
    enc = np.ascontiguousarray(encoder_output, dtype=np.float32)
    dec = np.ascontiguousarray(decoder_output, dtype=np.float32)
    assert enc.shape == (N, D) and dec.shape == (N, D)

    if "nc" not in _CACHE:
        _CACHE["nc"] = _build_program()
    nc = _CACHE["nc"]

    in_maps = []
    for r in range(NCORES):
        idx = (r * R + np.arange(WIN)) % N
        in_maps.append({
            "enc_slab": np.ascontiguousarray(enc[r * R:(r + 1) * R]),
            "dec_win": np.ascontiguousarray(dec[idx]),
        })
    res = run_bass_kernel_spmd(nc, in_maps, core_ids=list(range(NCORES)))

    sum_exp = np.zeros(N, dtype=np.float64)
    s_diag = np.empty(N, dtype=np.float64)
    for r in range(NCORES):
        acc = res.results[r]["acc_out"].astype(np.float64)      # [NWC, F]
        sum_exp += acc.reshape(N)                               # d = 512*wc + f
        sdr = res.results[r]["sdiag_out"].astype(np.float64)    # [P, NBJ]
        s_diag[r * R:(r + 1) * R] = sdr.T.reshape(R)            # j = 128*k + q

    h = -np.maximum(1.0 + M_M - s_diag, 0.0) * (s_diag - (1.0 - M_M)) * GAMMA
    hm = h.max()
    lse_p = hm + np.log(np.exp(h - hm).sum())
    L = np.log(sum_exp[1:])
    x = L + lse_p
    out = np.mean(np.log1p(np.exp(-np.abs(x))) + np.maximum(x, 0.0))
    return np.float32(out)



# revision 3
# speedup vs baseline: 1.6607x; 1.6607x over previous
"""CircleLoss Trainium2 kernel (8-core SPMD), v2.

Math: for S = cosine-sim(enc, dec) [N,N], both loss directions reduce to
per-wrapped-diagonal logsumexps of one matrix:
    out = mean_{d=1..N-1} softplus(L[d] + lse_p)
    L[d]  = log sum_j exp(g(S[j,(j+d)%N])),  g(s) = GAMMA*(max(s,-M)^2 - M^2)
    lse_p = logsumexp_j h(S[j,j])  (exact, computed on host)
g in [-4, 60] so sum(exp(g)) fits f32 with no max-pass.

Device chain per element: w = sqrt(GAMMA)*s from a f16 matmul (norms and
sqrt(GAMMA) folded into host-prepped operands), u = max(w,-2) via a 1-op
DVE tensor_scalar draining a wide 3-bank PSUM tile straight to f16 (runs
at accelerated rate), v = u*u via DVE tensor_tensor f16 2x (a slice routed
to ACT Square for balance), E = exp(v - 4) on ACT written as bf16.

Sharding: core r owns rows [1024r, 1024r+1024). Host pre-normalizes both
embeddings (f64) and ships f16 transposed operands. Each core computes its
1024 x 8320 sheared slab (row-tile bj reads dec window cols shifted by
128*bj so wrapped diagonals align across tiles: element (p, y) of every
tile has diagonal d = y - p), accumulates 4 row-tiles per quad in bf16,
bounces the two quad stripes through DRAM with a sheared re-read that turns
diagonals into columns, and column-sums via one-hot matmuls in PSUM. Host
sums the 8 per-core [8192] partials and finishes in float64.
"""

import numpy as np

import concourse.bass as bass
import concourse.bacc as bacc
import concourse.mybir as mybir
from concourse.tile import TileContext
from concourse.bass_utils import run_bass_kernel_spmd

N = 8192
D = 128
P = 128
NCORES = 8
R = N // NCORES          # 1024 rows per core
NBJ = R // P             # 8 row-tiles per core
WIN = 9216               # dec window columns per core
W2 = 8320                # sheared slab width (y = x - 128*bj, d = y - p)
NWC = 16                 # 512-wide d-chunks in the output
M_M = 0.25
GAMMA = 64.0
SQG = 8.0                # sqrt(GAMMA), folded into enc operand on host
EPS = 1e-5

# per-bj wide groups over y: 5 x 1536 + 1 x 640
GROUPS = [(0, 1536), (1536, 1536), (3072, 1536), (4608, 1536),
          (6144, 1536), (7680, 640)]

F32 = mybir.dt.float32
F16 = mybir.dt.float16
BF16 = mybir.dt.bfloat16

_CACHE = {}


def _build_program():
    nc = bacc.Bacc("TRN2", target_bir_lowering=False, debug=False,
                   num_devices=NCORES)
    encT = nc.dram_tensor("encT", [P, R], F16, kind="ExternalInput")
    decT = nc.dram_tensor("decT", [P, WIN], F16, kind="ExternalInput")
    acc_out = nc.dram_tensor("acc_out", [NWC, 512], F32, kind="ExternalOutput")
    stripes = nc.dram_tensor("stripes", [2, P, W2], BF16, kind="Internal")

    mx = mybir.AluOpType.max
    add = mybir.AluOpType.add
    AF = mybir.ActivationFunctionType

    with TileContext(nc) as tc:
        with (
            tc.tile_pool(name="persist", bufs=1) as persist,
            tc.tile_pool(name="mm", bufs=2, space="PSUM") as mmp,
            tc.tile_pool(name="apsum", bufs=1, space="PSUM") as apsum,
            tc.tile_pool(name="upool", bufs=3) as upool,
            tc.tile_pool(name="vpool", bufs=3) as vpool,
            tc.tile_pool(name="epool", bufs=3) as epool,
            tc.tile_pool(name="erpool", bufs=4) as erpool,
        ):
            enc_sb = persist.tile([P, R], F16)
            nc.sync.dma_start(out=enc_sb[:], in_=encT[:, :])
            dec_sb = persist.tile([P, WIN], F16)
            nc.sync.dma_start(out=dec_sb[:], in_=decT[:, :])

            onehot = persist.tile([P, NWC * NWC], BF16)
            bias_m4 = persist.tile([P, 1], F32)
            nc.vector.memset(bias_m4[:], -4.0)
            nc.gpsimd.memset(onehot[:], 0.0)
            for wc in range(NWC):
                nc.gpsimd.memset(onehot[:, wc * NWC + wc:wc * NWC + wc + 1],
                                 1.0)

            q0 = persist.tile([P, W2], BF16)
            q1 = persist.tile([P, W2], BF16)
            quads = [q0, q1]
            acc_ps = apsum.tile([NWC, 512], F32)
            nc.vector.memset(acc_ps[:], 0.0)
            acc_sb = persist.tile([NWC, 512], F32)

            # sheared reads eligible after stripe write of group g completes:
            # read rq covers stripe cols [1024rq, 1024rq+1150]
            reads_after = {0: [0], 1: [1], 2: [2, 3], 3: [4], 4: [5, 6],
                           5: [7]}

            for q in range(2):
                for bjl in range(4):
                    bj = 4 * q + bjl
                    for g, (y0, gw) in enumerate(GROUPS):
                        ps = mmp.tile([P, 1536], F32, tag="ps")
                        for c0 in range(0, gw, 512):
                            cw = min(512, gw - c0)
                            nc.tensor.matmul(
                                ps[:, c0:c0 + cw],
                                lhsT=enc_sb[:, bj * P:(bj + 1) * P],
                                rhs=dec_sb[:, 128 * bj + y0 + c0:
                                           128 * bj + y0 + c0 + cw],
                                start=True, stop=True)
                        ut = upool.tile([P, 1536], F16, tag="ut")
                        nc.vector.tensor_scalar(out=ut[:, 0:gw],
                                                in0=ps[:, 0:gw], scalar1=-2.0,
                                                scalar2=None, op0=mx)
                        vt = vpool.tile([P, 1536], F16, tag="vt")
                        if (bj + g) % 4 == 1:
                            nc.scalar.activation(vt[:, 0:gw], ut[:, 0:gw],
                                                 AF.Square)
                        else:
                            nc.vector.tensor_mul(vt[:, 0:gw], ut[:, 0:gw],
                                                 ut[:, 0:gw])
                        if bjl == 0:
                            nc.scalar.activation(quads[q][:, y0:y0 + gw],
                                                 vt[:, 0:gw], AF.Exp,
                                                 bias=bias_m4[:, 0:1],
                                                 scale=1.0)
                        else:
                            et = epool.tile([P, 1536], BF16, tag="et")
                            nc.scalar.activation(et[:, 0:gw], vt[:, 0:gw],
                                                 AF.Exp, bias=bias_m4[:, 0:1],
                                                 scale=1.0)
                            eng = nc.gpsimd if (bjl == 2 and g in (0, 2, 4)) \
                                else nc.vector
                            eng.tensor_tensor(
                                out=quads[q][:, y0:y0 + gw],
                                in0=et[:, 0:gw],
                                in1=quads[q][:, y0:y0 + gw], op=add)
                        if bjl == 3:
                            nc.sync.dma_start(
                                out=stripes[q, :, y0:y0 + gw],
                                in_=quads[q][:, y0:y0 + gw])
                            for rq in reads_after[g]:
                                er = erpool.tile([P, 1024], BF16, tag="er")
                                nc.sync.dma_start(
                                    out=er[:],
                                    in_=bass.AP(tensor=stripes,
                                                offset=q * P * W2 + 1024 * rq,
                                                ap=[[W2 + 1, P], [1, 1024]]))
                                for h in range(2):
                                    wc = 2 * rq + h
                                    nc.tensor.matmul(
                                        acc_ps[:],
                                        lhsT=onehot[:, wc * NWC:(wc + 1) * NWC],
                                        rhs=er[:, h * 512:(h + 1) * 512],
                                        start=False, stop=False,
                                        skip_group_check=True)
            nc.scalar.copy(acc_sb[:], acc_ps[:])
            nc.sync.dma_start(out=acc_out[:, :], in_=acc_sb[:])
    nc.compile()
    return nc


def make_in_maps(enc: np.ndarray, dec: np.ndarray):
    """Host prep: normalize in f64, fold sqrt(GAMMA) into enc, transpose,
    cast f16, build per-core window slices. Returns (in_maps, lse_p)."""
    e64 = enc.astype(np.float64)
    d64 = dec.astype(np.float64)
    en = np.sqrt((e64 * e64).sum(1, keepdims=True))
    dn = np.sqrt((d64 * d64).sum(1, keepdims=True))
    encn8 = (e64 / en * SQG).astype(np.float16)
    decn = (d64 / dn).astype(np.float16)

    s_jj = (e64 * d64).sum(1) / (en[:, 0] * dn[:, 0] + EPS)
    h = -np.maximum(1.0 + M_M - s_jj, 0.0) * (s_jj - (1.0 - M_M)) * GAMMA
    hm = h.max()
    lse_p = hm + np.log(np.exp(h - hm).sum())

    in_maps = []
    for r in range(NCORES):
        idx = (r * R + np.arange(WIN)) % N
        in_maps.append({
            "encT": np.ascontiguousarray(encn8[r * R:(r + 1) * R].T),
            "decT": np.ascontiguousarray(decn[idx].T),
        })
    return in_maps, lse_p


def kernel(encoder_output: np.ndarray, decoder_output: np.ndarray) -> np.ndarray:
    enc = np.ascontiguousarray(encoder_output, dtype=np.float32)
    dec = np.ascontiguousarray(decoder_output, dtype=np.float32)
    assert enc.shape == (N, D) and dec.shape == (N, D)

    if "nc" not in _CACHE:
        _CACHE["nc"] = _build_program()
    nc = _CACHE["nc"]

    in_maps, lse_p = make_in_maps(enc, dec)
    res = run_bass_kernel_spmd(nc, in_maps, core_ids=list(range(NCORES)))

    sum_exp = np.zeros(N, dtype=np.float64)
    for r in range(NCORES):
        acc = res.results[r]["acc_out"].astype(np.float64)      # [NWC, 512]
        sum_exp += acc.reshape(N)                               # d = 512*wc + f

    L = np.log(sum_exp[1:])
    x = L + lse_p
    out = np.mean(np.log1p(np.exp(-np.abs(x))) + np.maximum(x, 0.0))
    return np.float32(out)
